# Initial kernel scaffold
#
"""PointNet++ Feature Propagation kernel for Trainium2 (8 NeuronCores).

Math per batch element b (one NeuronCore each, data-parallel over B=8):
  1. d[n,s]   = ||xyz1[n]||^2 + ||xyz2[s]||^2 - 2 xyz1[n].xyz2[s]      [N,S]
  2. top-3 smallest d per n -> idx[n,3], weights w = (1/(d+eps)) / sum
  3. interp[n,:] = sum_j w_j * points2[idx_j,:]                        [N,D2]
  4. x = concat(points1, interp).T                                     [C,N]
  5. y1 = InstanceNorm(relu-after)(W1 @ x)   (biases cancel in IN)     [256,N]
  6. y2 = InstanceNorm(relu-after)(W2 @ y1)                            [128,N]
  7. out = y2.T                                                        [N,128]

Implementation notes:
 - distances via PE matmul with K=4 contraction using rows
   [x1x, x1y, x1z, 1] x [2*x2x, 2*x2y, 2*x2z, -||x2||^2], which computes
   p = 2 x1.x2 - ||x2||^2 = ||x1||^2 - d.  Per-row top-3 of -d == top-3 of p
   (row constant ||x1||^2 doesn't change per-row order); true d recovered as
   d_j = ||x1||^2 - vals_j on a tiny [128, NCH, 3] tensor.
 - top-8 values + indices per row via DVE InstMax / InstMaxIndex.
 - 3-NN rows of points2 fetched with gpsimd dma_gather (MoE-style row gather)
   straight from HBM; weighted-sum via per-partition tensor_scalar ops.
 - InstanceNorm biases b1/b2 are mathematically no-ops (per-channel shift is
   removed by the per-channel mean subtraction), so they are dropped.
 - InstanceNorm stats via bn_stats/bn_aggr; normalize+relu fused in one
   scalar-engine activation pass (scale=1/std, bias=-mean/std).
"""

import os
import sys

for _p in ("/opt/trn_rl_repo", "/root/.axon_site/_ro/trn_rl_repo"):
    if os.path.isdir(_p) and _p not in sys.path:
        sys.path.insert(0, _p)

import numpy as np

import concourse.bass as bass
import concourse.mybir as mybir
import concourse.tile as tile
from concourse import masks
from concourse.vector_clock import ScopedClock

F32 = mybir.dt.float32
U16 = mybir.dt.uint16
I16 = mybir.dt.int16
ALU = mybir.AluOpType
ACTF = mybir.ActivationFunctionType

EPS_INTERP = 1e-8
EPS_IN = 1e-5

_PATCHED = False


def _patch_tile_drain():
    """The pinned walrus rejects >N sync waits on the TileContext tail drain
    ("Too many sync wait commands").  Split the accumulated waits across
    single-wait nops on the sync engine before the barrier."""
    global _PATCHED
    if _PATCHED:
        return
    _PATCHED = True

    def _drain_and_barrier(self, tick_clock, wait_clock):
        nc = self.nc
        drain_inst = nc.sync.drain()
        wait_clock.add_sem_waits(
            drain_inst.ins, ScopedClock({None: tick_clock.global_clock})
        )
        waits = list(drain_inst.ins.sync_info.on_wait or [])
        if len(waits) > 1:
            drain_inst.ins.sync_info.on_wait = waits[:1]
            for w in waits[1:]:
                nop = nc.sync.nop(nofuse=True, hint="drain_wait_split")
                nop.ins.sync_info = mybir.SyncInfo(on_wait=[w], on_update=[])
        nc.all_engine_barrier()
        popped = nc._tile_sem_poison_stack.pop()
        assert popped is self._sem_poison
        nc.clear_and_free_semaphores(list(self.sems.allocated().values()))
        nc.all_engine_barrier()

    tile.TileContext._drain_and_barrier = _drain_and_barrier


def build_nc(N=8192, S=2048, D1=128, D2=256, C1O=256, C2O=128):
    """Emit the per-core program.  N: dense points, S: sparse points."""
    _patch_tile_drain()
    P = 128
    assert N % 1024 == 0 and S % 512 == 0 and D1 == 128 and D2 == 256
    NCH = N // P            # n-chunks of 128 rows
    SCH = S // 512          # 512-wide psum slices of the distance row
    NBLK = 1024             # n rows per gather block
    NBLKS = N // NBLK
    CPB = NBLK // P         # chunks per block
    CIN = D1 + D2
    KCH1 = CIN // P         # 3 contraction chunks for MLP1
    MCH1 = C1O // P         # 2 output chunks for MLP1
    KCH2 = C1O // P
    FN = N // 512           # 512-wide slices of N

    nc = bass.Bass()
    xyz1_d = nc.dram_tensor("xyz1", [N, 3], F32, kind="ExternalInput")
    xyz2_d = nc.dram_tensor("xyz2", [S, 3], F32, kind="ExternalInput")
    p1_d = nc.dram_tensor("points1", [N, D1], F32, kind="ExternalInput")
    p2_d = nc.dram_tensor("points2", [S, D2], F32, kind="ExternalInput")
    w1_d = nc.dram_tensor("W1", [C1O, CIN], F32, kind="ExternalInput")
    w2_d = nc.dram_tensor("W2", [C2O, C1O], F32, kind="ExternalInput")
    out_d = nc.dram_tensor("out", [N, C2O], F32, kind="ExternalOutput")
    jb_d = nc.dram_tensor("jb_scratch", [N, 3], I16)

    with tile.TileContext(nc) as tc:
        con = tc.tile_pool(name="con", bufs=1)
        tpp = tc.tile_pool(name="tpp", bufs=3, space=bass.MemorySpace.PSUM)
        ndp = tc.tile_pool(name="ndp", bufs=1, space=bass.MemorySpace.PSUM)
        mmp = tc.tile_pool(name="mmp", bufs=2, space=bass.MemorySpace.PSUM)
        psb = tc.tile_pool(name="psb", bufs=3)
        gat = tc.tile_pool(name="gat", bufs=1)
        sml = tc.tile_pool(name="sml", bufs=3)
        big = tc.tile_pool(name="big", bufs=1)

        with con, tpp, ndp, mmp, psb, gat, sml, big:
            ident = con.tile([P, P], F32)
            masks.make_identity(nc, ident[:])

            # ---------------- setup: distance operands ----------------
            # A4 = [x1x; x1y; x1z; 1]  (K=4 x N), lhsT for the distance matmul
            a4 = con.tile([4, N], F32)
            nc.sync.dma_start(a4[0:3, :], xyz1_d[:].rearrange("n c -> c n"))
            nc.vector.memset(a4[3:4, :], 1.0)

            # B4 = [2*x2x; 2*x2y; 2*x2z; -||x2||^2]  (K=4 x S), rhs
            x2t = con.tile([3, S], F32)
            nc.sync.dma_start(x2t[:], xyz2_d[:].rearrange("s c -> c s"))
            b4 = con.tile([4, S], F32)
            nc.scalar.mul(b4[0:3, :], x2t[:], 2.0)
            sq2 = con.tile([3, S], F32)
            nc.scalar.square(sq2[:], x2t[:])
            t01 = con.tile([1, S], F32)
            nc.vector.tensor_add(t01[:], sq2[0:1, :], sq2[1:2, :])
            # b4[3] = (t01 * -1) - sq2[2] = -(x^2+y^2+z^2)
            nc.vector.scalar_tensor_tensor(
                b4[3:4, :], t01[:], -1.0, sq2[2:3, :], op0=ALU.mult, op1=ALU.subtract
            )

            # n1sq[p, c] = ||xyz1[c*128+p]||^2  (+eps variant for weights)
            x1n = con.tile([P, NCH, 3], F32)
            nc.sync.dma_start(
                x1n[:], xyz1_d[:].rearrange("(c p) x -> p c x", p=P)
            )
            x1sq = con.tile([P, NCH, 3], F32)
            nc.vector.tensor_mult(x1sq[:], x1n[:], x1n[:])
            n1sq = con.tile([P, NCH], F32)
            nc.vector.tensor_reduce(
                n1sq[:], x1sq[:], axis=mybir.AxisListType.X, op=ALU.add
            )
            n1sqe = con.tile([P, NCH], F32)
            nc.vector.tensor_scalar_add(n1sqe[:], n1sq[:], float(EPS_INTERP))

            # ---------------- weights W1^T, W2^T via PE transpose ----------------
            w1sb = con.tile([P, MCH1, CIN], F32)
            for m in range(MCH1):
                nc.sync.dma_start(w1sb[:, m, :], w1_d[m * P:(m + 1) * P, :])
            w1t = con.tile([P, KCH1, C1O], F32)
            for kc in range(KCH1):
                for m in range(MCH1):
                    pst = tpp.tile([P, P], F32, tag="tp")
                    nc.tensor.transpose(
                        pst[:], w1sb[:, m, kc * P:(kc + 1) * P], ident[:]
                    )
                    nc.any.tensor_copy(w1t[:, kc, m * P:(m + 1) * P], pst[:])
            w2sb = con.tile([P, C1O], F32)
            nc.sync.dma_start(w2sb[:], w2_d[:])
            w2t = con.tile([P, KCH2, C2O], F32)
            for kc in range(KCH2):
                pst = tpp.tile([P, P], F32, tag="tp")
                nc.tensor.transpose(
                    pst[:], w2sb[:, kc * P:(kc + 1) * P], ident[:]
                )
                nc.any.tensor_copy(w2t[:, kc, :], pst[:])

            # ---------------- per-row top-8 buffers ----------------
            vals = con.tile([P, NCH, 8], F32)
            idx8 = con.tile([P, NCH, 8], U16)
            r3 = con.tile([P, NCH, 3], F32)
            w3 = con.tile([P, NCH, 3], F32)
            zs = con.tile([P, NCH], F32)
            zi = con.tile([P, NCH], F32)

            # interp^T halves, built incrementally: [128, N] each
            it0 = big.tile([P, N], F32, tag="it0")
            it1 = big.tile([P, N], F32, tag="it1")

            # ---------------- phase B+C: distances, top3, gather, wsum ----------
            for blk in range(NBLKS):
                for cc in range(CPB):
                    c = blk * CPB + cc
                    nd = ndp.tile([P, S], F32, tag="nd")
                    for sc in range(SCH):
                        nc.tensor.matmul(
                            nd[:, sc * 512:(sc + 1) * 512],
                            a4[:, c * P:(c + 1) * P],
                            b4[:, sc * 512:(sc + 1) * 512],
                            start=True,
                            stop=True,
                        )
                    pb = psb.tile([P, S], F32, tag="pb")
                    for sc in range(SCH):
                        nc.any.tensor_copy(
                            pb[:, sc * 512:(sc + 1) * 512],
                            nd[:, sc * 512:(sc + 1) * 512],
                        )
                    nc.vector.max(vals[:, c, :], pb[:])
                    nc.vector.max_index(idx8[:, c, :], vals[:, c, :], pb[:])

                bsl = slice(blk * CPB, (blk + 1) * CPB)
                # r3 = 1/(d+eps) with d_j = n1sq - vals_j  (top-3 only)
                nc.vector.scalar_tensor_tensor(
                    r3[:, bsl, :],
                    vals[:, bsl, 0:3],
                    -1.0,
                    n1sqe[:, bsl].to_broadcast([P, CPB, 3]),
                    op0=ALU.mult,
                    op1=ALU.add,
                )
                nc.vector.reciprocal(r3[:, bsl, :], r3[:, bsl, :])
                nc.vector.tensor_reduce(
                    zs[:, bsl], r3[:, bsl, :], axis=mybir.AxisListType.X, op=ALU.add
                )
                nc.vector.reciprocal(zi[:, bsl], zs[:, bsl])
                nc.vector.tensor_mult(
                    w3[:, bsl, :], r3[:, bsl, :],
                    zi[:, bsl].to_broadcast([P, CPB, 3]),
                )

                # indices: bounce through DRAM into the 16-partition wrap layout
                nc.sync.dma_start(
                    jb_d[blk * NBLK:(blk + 1) * NBLK, :].rearrange(
                        "(cc p) j -> p cc j", p=P
                    ),
                    idx8[:, bsl, 0:3].bitcast(I16),
                )
                gtiles = []
                for j in range(3):
                    idxw = sml.tile([P, NBLK // 16], I16, tag=f"idxw{j}")
                    nc.vector.memset(idxw[:], 0)
                    nc.sync.dma_start(
                        idxw[0:16, :],
                        jb_d[blk * NBLK:(blk + 1) * NBLK, j:j + 1].rearrange(
                            "(col p) one -> p (col one)", p=16
                        ),
                    )
                    g = gat.tile([P, CPB, D2], F32, tag=f"g{j}")
                    nc.gpsimd.dma_gather(
                        out_ap=g[:],
                        in_ap=p2_d[:],
                        idxs_ap=idxw[:],
                        num_idxs=NBLK,
                        num_idxs_reg=NBLK,
                        elem_size=D2,
                    )
                    gtiles.append(g)

                for cc in range(CPB):
                    c = blk * CPB + cc
                    it = sml.tile([P, D2], F32, tag="interp")
                    nc.vector.tensor_scalar_mul(
                        it[:], gtiles[0][:, cc, :], w3[:, c, 0:1]
                    )
                    for j in (1, 2):
                        nc.vector.scalar_tensor_tensor(
                            it[:], gtiles[j][:, cc, :], w3[:, c, j:j + 1], it[:],
                            op0=ALU.mult, op1=ALU.add,
                        )
                    for h, itt in ((0, it0), (1, it1)):
                        pst = tpp.tile([P, P], F32, tag="tp")
                        nc.tensor.transpose(
                            pst[:], it[:, h * P:(h + 1) * P], ident[:]
                        )
                        nc.any.tensor_copy(itt[:, c * P:(c + 1) * P], pst[:])

            # ---------------- phase D: MLP1 + stats ----------------
            y1c = [big.tile([P, N], F32, tag=f"y1_{m}") for m in range(MCH1)]
            st1 = con.tile([P, MCH1, FN, 6], F32)
            for f in range(FN):
                fsl = slice(f * 512, (f + 1) * 512)
                p1c = sml.tile([P, 512], F32, tag="p1c")
                for q in range(4):
                    nch = f * 4 + q
                    p1in = sml.tile([P, P], F32, tag="p1in")
                    nc.sync.dma_start(p1in[:], p1_d[nch * P:(nch + 1) * P, :])
                    pst = tpp.tile([P, P], F32, tag="tp")
                    nc.tensor.transpose(pst[:], p1in[:], ident[:])
                    nc.any.tensor_copy(p1c[:, q * P:(q + 1) * P], pst[:])
                rhs = [p1c[:], it0[:, fsl], it1[:, fsl]]
                for m in range(MCH1):
                    yp = mmp.tile([P, 512], F32, tag="mm")
                    for kc in range(KCH1):
                        nc.tensor.matmul(
                            yp[:],
                            w1t[:, kc, m * P:(m + 1) * P],
                            rhs[kc],
                            start=(kc == 0),
                            stop=(kc == KCH1 - 1),
                        )
                    nc.any.tensor_copy(y1c[m][:, fsl], yp[:])
                    nc.vector.bn_stats(st1[:, m, f, :], yp[:])

            mv1 = con.tile([P, MCH1, 2], F32)
            inv1 = con.tile([P, MCH1], F32)
            nb1 = con.tile([P, MCH1], F32)
            for m in range(MCH1):
                nc.vector.bn_aggr(mv1[:, m, :], st1[:, m, :, :])
                # 1/sqrt(var+eps), then bias = -mean/std
                nc.scalar.activation(
                    inv1[:, m:m + 1], mv1[:, m, 1:2], ACTF.Sqrt, bias=float(EPS_IN)
                )
                nc.vector.reciprocal(inv1[:, m:m + 1], inv1[:, m:m + 1])
                nc.vector.scalar_tensor_tensor(
                    nb1[:, m:m + 1], mv1[:, m, 0:1], -1.0, inv1[:, m:m + 1],
                    op0=ALU.mult, op1=ALU.mult,
                )
                nc.scalar.activation(
                    y1c[m][:], y1c[m][:], ACTF.Relu,
                    bias=nb1[:, m:m + 1], scale=inv1[:, m:m + 1],
                )

            # ---------------- phase E: MLP2 + stats + out ----------------
            y2 = big.tile([P, N], F32, tag="it0")  # reuses it0's slot
            st2 = con.tile([P, FN, 6], F32)
            for f in range(FN):
                fsl = slice(f * 512, (f + 1) * 512)
                yp = mmp.tile([P, 512], F32, tag="mm")
                for kc in range(KCH2):
                    nc.tensor.matmul(
                        yp[:],
                        w2t[:, kc, :],
                        y1c[kc][:, fsl],
                        start=(kc == 0),
                        stop=(kc == KCH2 - 1),
                    )
                nc.any.tensor_copy(y2[:, fsl], yp[:])
                nc.vector.bn_stats(st2[:, f, :], yp[:])

            mv2 = con.tile([P, 2], F32)
            inv2 = con.tile([P, 1], F32)
            nb2 = con.tile([P, 1], F32)
            nc.vector.bn_aggr(mv2[:], st2[:])
            nc.scalar.activation(inv2[:], mv2[:, 1:2], ACTF.Sqrt, bias=float(EPS_IN))
            nc.vector.reciprocal(inv2[:], inv2[:])
            nc.vector.scalar_tensor_tensor(
                nb2[:], mv2[:, 0:1], -1.0, inv2[:], op0=ALU.mult, op1=ALU.mult
            )
            nc.scalar.activation(
                y2[:], y2[:], ACTF.Relu, bias=nb2[:], scale=inv2[:]
            )

            for f in range(FN):
                ot = sml.tile([P, 4, C2O], F32, tag="ot")
                for q in range(4):
                    nch = f * 4 + q
                    pst = tpp.tile([P, P], F32, tag="tp")
                    nc.tensor.transpose(
                        pst[:], y2[:, nch * P:(nch + 1) * P], ident[:]
                    )
                    nc.any.tensor_copy(ot[:, q, :], pst[:])
                nc.sync.dma_start(
                    out_d[f * 512:(f + 1) * 512, :].rearrange(
                        "(q p) o -> p q o", p=P
                    ),
                    ot[:],
                )

    return nc


_NC_CACHE = {}


def _get_nc(key=(8192, 2048)):
    if key not in _NC_CACHE:
        _NC_CACHE[key] = build_nc(N=key[0], S=key[1])
    return _NC_CACHE[key]


def kernel(**inputs):
    from concourse.bass_utils import run_bass_kernel_spmd

    B = inputs["xyz1"].shape[0]
    assert B == 8
    nc = _get_nc()
    names = ("xyz1", "xyz2", "points1", "points2")
    in_maps = []
    for b in range(B):
        m = {k: np.ascontiguousarray(inputs[k][b], dtype=np.float32) for k in names}
        m["W1"] = np.ascontiguousarray(inputs["W1"], dtype=np.float32)
        m["W2"] = np.ascontiguousarray(inputs["W2"], dtype=np.float32)
        in_maps.append(m)
    res = run_bass_kernel_spmd(nc, in_maps, core_ids=list(range(B)))
    return np.stack([res.results[b]["out"] for b in range(B)])


# revision 49
# speedup vs baseline: 1.3172x; 1.3172x over previous
"""PointNet++ Feature Propagation kernel for Trainium2 (8 NeuronCores).

Math per batch element b (one NeuronCore each, data-parallel over B=8):
  1. d[n,s]   = ||xyz1[n]||^2 + ||xyz2[s]||^2 - 2 xyz1[n].xyz2[s]      [N,S]
  2. top-3 smallest d per n -> idx[n,3], weights w = (1/(d+eps)) / sum
  3. interp[n,:] = sum_j w_j * points2[idx_j,:]                        [N,D2]
  4. x = concat(points1, interp).T                                     [C,N]
  5. y1 = InstanceNorm(relu-after)(W1 @ x)   (biases cancel in IN)     [256,N]
  6. y2 = InstanceNorm(relu-after)(W2 @ y1)                            [128,N]
  7. out = y2.T                                                        [N,128]

Implementation notes:
 - distances via PE matmul with K=4 contraction using rows
   [x1x, x1y, x1z, 1] x [2*x2x, 2*x2y, 2*x2z, -||x2||^2], which computes
   p = 2 x1.x2 - ||x2||^2 = ||x1||^2 - d.  Per-row top-3 of -d == top-3 of p
   (row constant ||x1||^2 doesn't change per-row order); true d recovered as
   d_j = ||x1||^2 - vals_j on a tiny [128, NCH, 3] tensor.
 - top-8 values + indices per row via DVE InstMax / InstMaxIndex.
 - 3-NN rows of points2 fetched with gpsimd dma_gather (MoE-style row gather)
   straight from HBM; weighted-sum via per-partition tensor_scalar ops.
 - InstanceNorm biases b1/b2 are mathematically no-ops (per-channel shift is
   removed by the per-channel mean subtraction), so they are dropped.
 - InstanceNorm stats via bn_stats/bn_aggr; normalize+relu fused in one
   scalar-engine activation pass (scale=1/std, bias=-mean/std).
"""

import os
import sys

for _p in ("/opt/trn_rl_repo", "/root/.axon_site/_ro/trn_rl_repo"):
    if os.path.isdir(_p) and _p not in sys.path:
        sys.path.insert(0, _p)

import numpy as np

import concourse.bass as bass
import concourse.bacc as bacc
import concourse.mybir as mybir
import concourse.tile as tile
from concourse import masks
from concourse.vector_clock import ScopedClock

F32 = mybir.dt.float32
U16 = mybir.dt.uint16
I16 = mybir.dt.int16
ALU = mybir.AluOpType
ACTF = mybir.ActivationFunctionType

EPS_INTERP = 1e-8
EPS_IN = 1e-5

_PATCHED = False


def _patch_tile_drain():
    """The pinned walrus rejects >N sync waits on the TileContext tail drain
    ("Too many sync wait commands").  Split the accumulated waits across
    single-wait nops on the sync engine before the barrier."""
    global _PATCHED
    if _PATCHED:
        return
    _PATCHED = True

    def _drain_and_barrier(self, tick_clock, wait_clock):
        nc = self.nc
        drain_inst = nc.sync.drain()
        wait_clock.add_sem_waits(
            drain_inst.ins, ScopedClock({None: tick_clock.global_clock})
        )
        si = drain_inst.ins.sync_info
        waits = list(si.on_wait or []) if si else []
        if len(waits) > 1:
            drain_inst.ins.sync_info.on_wait = waits[:1]
            for w in waits[1:]:
                nop = nc.sync.nop(nofuse=True, hint="drain_wait_split")
                nop.ins.sync_info = mybir.SyncInfo(on_wait=[w], on_update=[])
        nc.all_engine_barrier()
        popped = nc._tile_sem_poison_stack.pop()
        assert popped is self._sem_poison
        nc.clear_and_free_semaphores(list(self.sems.allocated().values()))
        nc.all_engine_barrier()

    tile.TileContext._drain_and_barrier = _drain_and_barrier


def build_nc(N=8192, S=2048, D1=128, D2=256, C1O=256, C2O=128, debug=False,
             repeat=1):
    """Emit the per-core program.  N: dense points, S: sparse points."""
    _patch_tile_drain()
    P = 128
    assert N % 1024 == 0 and S % 512 == 0 and D1 == 128 and D2 == 256
    NCH = N // P            # n-chunks of 128 rows
    SCH = S // 512          # 512-wide psum slices of the distance row
    NBLK = 512              # n rows per gather / distance block
    NBLKS = N // NBLK
    CPB = NBLK // P         # chunks per block
    CIN = D1 + D2
    KCH1 = CIN // P         # 3 contraction chunks for MLP1
    MCH1 = C1O // P         # 2 output chunks for MLP1
    KCH2 = C1O // P
    FN = N // 512           # 512-wide slices of N

    nc = bacc.Bacc(None, target_bir_lowering=False)
    xyz1_d = nc.dram_tensor("xyz1", [N, 3], F32, kind="ExternalInput")
    xyz2_d = nc.dram_tensor("xyz2", [S, 3], F32, kind="ExternalInput")
    p1_d = nc.dram_tensor("points1", [N, D1], F32, kind="ExternalInput")
    p2_d = nc.dram_tensor("points2", [S, D2], F32, kind="ExternalInput")
    w1_d = nc.dram_tensor("W1", [C1O, CIN], F32, kind="ExternalInput")
    w2_d = nc.dram_tensor("W2", [C2O, C1O], F32, kind="ExternalInput")
    out_d = nc.dram_tensor("out", [N, C2O], F32, kind="ExternalOutput")
    if debug:
        dbg = {
            "dbg_pb": nc.dram_tensor("dbg_pb", [P, S], F32, kind="ExternalOutput"),
            "dbg_vals": nc.dram_tensor(
                "dbg_vals", [P, N // P, 8], F32, kind="ExternalOutput"
            ),
            "dbg_idx": nc.dram_tensor(
                "dbg_idx", [P, N // P, 8], U16, kind="ExternalOutput"
            ),
            "dbg_w3": nc.dram_tensor(
                "dbg_w3", [P, N // P, 3], F32, kind="ExternalOutput"
            ),
            "dbg_g0": nc.dram_tensor(
                "dbg_g0", [P, 4, D2], F32, kind="ExternalOutput"
            ),
            "dbg_it0": nc.dram_tensor("dbg_it0", [P, N], F32, kind="ExternalOutput"),
            "dbg_y1": nc.dram_tensor("dbg_y1", [P, N], F32, kind="ExternalOutput"),
        }
    jb_d = nc.dram_tensor("jb_scratch", [3, N], I16)
    ns2_d = nc.dram_tensor("ns2_scratch", [S], F32)
    x1t_d = nc.dram_tensor("x1t_scratch", [3, N], F32)

    from contextlib import ExitStack

    with tile.TileContext(nc) as tc, ExitStack() as es:
        con = es.enter_context(tc.tile_pool(name="con", bufs=1))
        tpp = es.enter_context(
            tc.tile_pool(name="tpp", bufs=2, space=bass.MemorySpace.PSUM)
        )
        ndp = es.enter_context(
            tc.tile_pool(name="ndp", bufs=1, space=bass.MemorySpace.PSUM)
        )
        mmp = es.enter_context(
            tc.tile_pool(name="mmp", bufs=2, space=bass.MemorySpace.PSUM)
        )
        psb = es.enter_context(tc.tile_pool(name="psb", bufs=2))
        gat = es.enter_context(tc.tile_pool(name="gat", bufs=1))
        sml = es.enter_context(tc.tile_pool(name="sml", bufs=3))
        big = es.enter_context(tc.tile_pool(name="big", bufs=1))

        from contextlib import nullcontext

        with tc.For_i(0, repeat, 1) if repeat > 1 else nullcontext():
            ident = con.tile([P, P], F32)
            masks.make_identity(nc, ident[:])

            # ---------------- setup: distance operands ----------------
            # matmul computes p' = x1.x2 - ||x2||^2/2 = (||x1||^2 - d)/2.
            # Row order puts the constant/norm row at partition 0 so every
            # compute op starts at partition 0 (DVE/ACT start-partition rule);
            # coordinate rows 1..3 are raw DMA loads.
            # A4 = [1; x1x; x1y; x1z] (K=4 x NBLK), lhsT, streamed per block.
            # One strided transpose pass DRAM->DRAM so the per-block loads are
            # contiguous.
            with nc.allow_non_contiguous_dma(reason="one-time xyz transpose"):
                nc.sync.dma_start(x1t_d[:], xyz1_d[:].rearrange("n c -> c n"))
            # B4 = [-||x2||^2/2; x2x; x2y; x2z]  (K=4 x S), rhs
            b4 = con.tile([4, S], F32)
            with nc.allow_non_contiguous_dma(reason="one-time xyz transpose"):
                nc.sync.dma_start(b4[1:4, :], xyz2_d[:].rearrange("s c -> c s"))
            # -||x2||^2/2 computed in [128, S/128] layout, bounced via DRAM
            x2n = con.tile([P, S // P, 3], F32)
            nc.sync.dma_start(x2n[:], xyz2_d[:].rearrange("(c p) x -> p c x", p=P))
            x2sq = con.tile([P, S // P, 3], F32)
            nc.vector.tensor_mul(x2sq[:], x2n[:], x2n[:])
            n2sq = con.tile([P, S // P], F32)
            nc.vector.tensor_reduce(
                n2sq[:], x2sq[:], axis=mybir.AxisListType.X, op=ALU.add
            )
            n2sqh = con.tile([P, S // P], F32)
            nc.vector.tensor_scalar_mul(n2sqh[:], n2sq[:], -0.5)
            nc.sync.dma_start(ns2_d[:].rearrange("(c p) -> p c", p=P), n2sqh[:])
            nc.sync.dma_start(b4[0:1, :], ns2_d[:].rearrange("s -> () s"))

            # n1sq[p, c] = ||xyz1[c*128+p]||^2  (+eps variant for weights)
            x1n = con.tile([P, NCH, 3], F32)
            nc.sync.dma_start(
                x1n[:], xyz1_d[:].rearrange("(c p) x -> p c x", p=P)
            )
            x1sq = con.tile([P, NCH, 3], F32)
            nc.vector.tensor_mul(x1sq[:], x1n[:], x1n[:])
            n1sq = con.tile([P, NCH], F32)
            nc.vector.tensor_reduce(
                n1sq[:], x1sq[:], axis=mybir.AxisListType.X, op=ALU.add
            )
            n1sqe = con.tile([P, NCH], F32)
            nc.vector.tensor_scalar_add(n1sqe[:], n1sq[:], float(EPS_INTERP))

            # ---------------- weights W1^T, W2^T via PE transpose ----------------
            w1sb = con.tile([P, MCH1, CIN], F32)
            for m in range(MCH1):
                nc.sync.dma_start(w1sb[:, m, :], w1_d[m * P:(m + 1) * P, :])
            w1t = con.tile([P, KCH1, C1O], F32)
            for kc in range(KCH1):
                for m in range(MCH1):
                    pst = tpp.tile([P, P], F32, tag="tp")
                    nc.tensor.transpose(
                        pst[:], w1sb[:, m, kc * P:(kc + 1) * P], ident[:]
                    )
                    nc.any.tensor_copy(w1t[:, kc, m * P:(m + 1) * P], pst[:])
            w2sb = con.tile([P, C1O], F32)
            nc.sync.dma_start(w2sb[:], w2_d[:])
            w2t = con.tile([P, KCH2, C2O], F32)
            for kc in range(KCH2):
                pst = tpp.tile([P, P], F32, tag="tp")
                nc.tensor.transpose(
                    pst[:], w2sb[:, kc * P:(kc + 1) * P], ident[:]
                )
                nc.any.tensor_copy(w2t[:, kc, :], pst[:])

            # ---------------- per-row top-8 buffers ----------------
            vals = con.tile([P, NCH, 8], F32)
            idx8 = con.tile([P, NCH, 8], U16)
            r3i = con.tile([P, NCH, 3], F32)
            r3 = con.tile([P, NCH, 3], F32)
            w3 = con.tile([P, NCH, 3], F32)
            zs = con.tile([P, NCH], F32)
            zi = con.tile([P, NCH], F32)

            # interp^T halves, built incrementally: [128, N] each
            it0 = big.tile([P, N], F32, tag="it0")
            it1 = big.tile([P, N], F32, tag="it1")
            y1c = [
                big.tile([P, N], F32, tag=f"y1_{m}", name=f"y1_{m}")
                for m in range(MCH1)
            ]
            s1y = con.tile([P, MCH1, FN], F32)
            s1q = con.tile([P, MCH1, FN], F32)

            # ---------------- phase B+C: distances, top3, gather, wsum ----------
            assert NBLKS == FN, "block loop doubles as the MLP1 f loop"
            for blk in range(NBLKS):
                a4b = sml.tile([4, NBLK], F32, tag="a4b", bufs=2)
                nc.gpsimd.memset(a4b[0:1, :], 1.0)
                nc.sync.dma_start(
                    a4b[1:4, :], x1t_d[:, blk * NBLK:(blk + 1) * NBLK]
                )
                for cc in range(CPB):
                    c = blk * CPB + cc
                    nd = ndp.tile([P, S], F32, tag="nd")
                    for sc in range(SCH):
                        nc.tensor.matmul(
                            nd[:, sc * 512:(sc + 1) * 512],
                            a4b[:, cc * P:(cc + 1) * P],
                            b4[:, sc * 512:(sc + 1) * 512],
                            start=True,
                            stop=True,
                        )
                    pb = psb.tile([P, S], F32, tag="pb")
                    for sc in range(SCH):
                        nc.any.tensor_copy(
                            pb[:, sc * 512:(sc + 1) * 512],
                            nd[:, sc * 512:(sc + 1) * 512],
                        )
                    nc.vector.max(vals[:, c, :], pb[:])
                    nc.vector.max_index(idx8[:, c, :], vals[:, c, :], pb[:])
                    if debug and c == 0:
                        nc.sync.dma_start(dbg["dbg_pb"][:], pb[:])

                bsl = slice(blk * CPB, (blk + 1) * CPB)
                # r3 = 1/(d+eps) with d_j = n1sq - 2*vals_j  (top-3 only).
                # Clamp positive: fp32 rounding can push a near-zero d+eps
                # negative, and 1/x of that poisons the weights.
                nc.vector.scalar_tensor_tensor(
                    r3i[:, bsl, :],
                    vals[:, bsl, 0:3],
                    -2.0,
                    n1sqe[:, bsl].to_broadcast([P, CPB, 3]),
                    op0=ALU.mult,
                    op1=ALU.add,
                )
                nc.vector.tensor_scalar_max(
                    r3i[:, bsl, :], r3i[:, bsl, :], float(EPS_INTERP)
                )
                nc.vector.reciprocal(r3[:, bsl, :], r3i[:, bsl, :])
                nc.vector.tensor_reduce(
                    zs[:, bsl], r3[:, bsl, :], axis=mybir.AxisListType.X, op=ALU.add
                )
                nc.vector.reciprocal(zi[:, bsl], zs[:, bsl])
                nc.vector.tensor_mul(
                    w3[:, bsl, :], r3[:, bsl, :],
                    zi[:, bsl].to_broadcast([P, CPB, 3]),
                )

                # indices: bounce through DRAM (j-major) into the 16-partition
                # wrap layout; per-j loads are then contiguous 1KB reads
                with nc.allow_non_contiguous_dma(reason="idx wrap bounce"):
                    for j in range(3):
                        nc.sync.dma_start(
                            jb_d[j, blk * NBLK:(blk + 1) * NBLK].rearrange(
                                "(cc p) -> p cc", p=P
                            ),
                            idx8[:, bsl, j].bitcast(I16),
                        )
                gtiles = []
                for j in range(3):
                    idxw = sml.tile([P, NBLK // 16], I16, tag=f"idxw{j}")
                    nc.gpsimd.memset(idxw[:], 0)
                    nc.sync.dma_start(
                        idxw[0:16, :],
                        jb_d[j, blk * NBLK:(blk + 1) * NBLK].rearrange(
                            "(col p) -> p col", p=16
                        ),
                    )
                    g = gat.tile([P, CPB, D2], F32, tag=f"g{j}")
                    nc.gpsimd.dma_gather(
                        out_ap=g[:],
                        in_ap=p2_d[:],
                        idxs_ap=idxw[:],
                        num_idxs=NBLK,
                        num_idxs_reg=NBLK,
                        elem_size=D2,
                    )
                    gtiles.append(g)
                if debug and blk == 0:
                    nc.sync.dma_start(dbg["dbg_g0"][:], gtiles[0][:])

                for cc in range(CPB):
                    c = blk * CPB + cc
                    it = sml.tile([P, D2], F32, tag="interp")
                    nc.scalar.activation(
                        it[:], gtiles[0][:, cc, :], ACTF.Copy,
                        scale=w3[:, c, 0:1],
                    )
                    for j in (1, 2):
                        nc.vector.scalar_tensor_tensor(
                            it[:], gtiles[j][:, cc, :], w3[:, c, j:j + 1], it[:],
                            op0=ALU.mult, op1=ALU.add,
                        )
                    for h, itt in ((0, it0), (1, it1)):
                        pst = tpp.tile([P, P], F32, tag="tp")
                        nc.tensor.transpose(
                            pst[:], it[:, h * P:(h + 1) * P], ident[:]
                        )
                        nc.any.tensor_copy(itt[:, c * P:(c + 1) * P], pst[:])

                # ---- MLP1 for the f-slice this block just finished ----
                f = blk
                fsl = slice(f * 512, (f + 1) * 512)
                p1f = sml.tile([P, 4, D1], F32, tag="p1f", bufs=2)
                nc.sync.dma_start(
                    p1f[:],
                    p1_d[f * 512:(f + 1) * 512, :].rearrange(
                        "(q p) d -> p q d", p=P
                    ),
                )
                p1c = sml.tile([P, 512], F32, tag="p1c", bufs=2)
                for q in range(4):
                    pst = tpp.tile([P, P], F32, tag="tp")
                    nc.tensor.transpose(pst[:], p1f[:, q, :], ident[:])
                    nc.any.tensor_copy(p1c[:, q * P:(q + 1) * P], pst[:])
                rhs = [p1c[:], it0[:, fsl], it1[:, fsl]]
                for m in range(MCH1):
                    yp = mmp.tile([P, 512], F32, tag="mm")
                    for kc in range(KCH1):
                        nc.tensor.matmul(
                            yp[:],
                            w1t[:, kc, m * P:(m + 1) * P],
                            rhs[kc],
                            start=(kc == 0),
                            stop=(kc == KCH1 - 1),
                        )
                    # copy + per-slice sums via the ACT accumulator (bn_stats
                    # HW semantics proved untrustworthy)
                    nc.scalar.activation(
                        y1c[m][:, fsl], yp[:], ACTF.Copy,
                        accum_out=s1y[:, m, f:f + 1],
                    )
                    sqs = sml.tile([P, 512], F32, tag="sqs", bufs=2)
                    nc.scalar.activation(
                        sqs[:], yp[:], ACTF.Square, accum_out=s1q[:, m, f:f + 1]
                    )

            if debug:
                nc.sync.dma_start(dbg["dbg_vals"][:], vals[:])
                nc.sync.dma_start(dbg["dbg_idx"][:], idx8[:])
                nc.sync.dma_start(dbg["dbg_w3"][:], w3[:])
                nc.sync.dma_start(dbg["dbg_it0"][:], it0[:])

            # ---------------- norm1 (after all blocks) ----------------
            # mean = sum(y)/N; var = sum(y^2)/N - mean^2 (biased, torch-style)
            epsn = con.tile([P, 1], F32)
            nc.vector.memset(epsn[:], float(EPS_IN))
            mean1 = con.tile([P, MCH1], F32)
            msq1 = con.tile([P, MCH1], F32)
            var1 = con.tile([P, MCH1], F32)
            std1 = con.tile([P, MCH1], F32)
            inv1 = con.tile([P, MCH1], F32)
            nb1 = con.tile([P, MCH1], F32)
            nc.vector.tensor_reduce(
                mean1[:], s1y[:], axis=mybir.AxisListType.X, op=ALU.add
            )
            nc.vector.tensor_scalar_mul(mean1[:], mean1[:], 1.0 / N)
            nc.vector.tensor_reduce(
                msq1[:], s1q[:], axis=mybir.AxisListType.X, op=ALU.add
            )
            nc.vector.tensor_scalar_mul(msq1[:], msq1[:], 1.0 / N)
            # var = msq - mean*mean
            nc.vector.tensor_mul(var1[:], mean1[:], mean1[:])
            nc.vector.tensor_sub(var1[:], msq1[:], var1[:])
            nc.scalar.activation(std1[:], var1[:], ACTF.Sqrt, bias=epsn[:])
            nc.vector.reciprocal(inv1[:], std1[:])
            nc.vector.scalar_tensor_tensor(
                nb1[:], mean1[:], -1.0, inv1[:], op0=ALU.mult, op1=ALU.mult
            )

            # ---------------- phase E: norm1+MLP2+stats, per f-slice --------
            y2 = big.tile([P, N], F32, tag="it0")  # reuses it0's slot
            s2y = con.tile([P, FN], F32)
            s2q = con.tile([P, FN], F32)
            for f in range(FN):
                fsl = slice(f * 512, (f + 1) * 512)
                for m in range(MCH1):
                    nc.scalar.activation(
                        y1c[m][:, fsl], y1c[m][:, fsl], ACTF.Relu,
                        bias=nb1[:, m:m + 1], scale=inv1[:, m:m + 1],
                    )
                yp = mmp.tile([P, 512], F32, tag="mm")
                for kc in range(KCH2):
                    nc.tensor.matmul(
                        yp[:],
                        w2t[:, kc, :],
                        y1c[kc][:, fsl],
                        start=(kc == 0),
                        stop=(kc == KCH2 - 1),
                    )
                nc.scalar.activation(
                    y2[:, fsl], yp[:], ACTF.Copy, accum_out=s2y[:, f:f + 1]
                )
                sqs = sml.tile([P, 512], F32, tag="sqs", bufs=2)
                nc.scalar.activation(
                    sqs[:], yp[:], ACTF.Square, accum_out=s2q[:, f:f + 1]
                )
            if debug:
                nc.sync.dma_start(dbg["dbg_y1"][:], y1c[0][:])

            mean2 = con.tile([P, 1], F32)
            msq2 = con.tile([P, 1], F32)
            var2 = con.tile([P, 1], F32)
            std2 = con.tile([P, 1], F32)
            inv2 = con.tile([P, 1], F32)
            nb2 = con.tile([P, 1], F32)
            nc.vector.tensor_reduce(
                mean2[:], s2y[:], axis=mybir.AxisListType.X, op=ALU.add
            )
            nc.vector.tensor_scalar_mul(mean2[:], mean2[:], 1.0 / N)
            nc.vector.tensor_reduce(
                msq2[:], s2q[:], axis=mybir.AxisListType.X, op=ALU.add
            )
            nc.vector.tensor_scalar_mul(msq2[:], msq2[:], 1.0 / N)
            nc.vector.tensor_mul(var2[:], mean2[:], mean2[:])
            nc.vector.tensor_sub(var2[:], msq2[:], var2[:])
            nc.scalar.activation(std2[:], var2[:], ACTF.Sqrt, bias=epsn[:])
            nc.vector.reciprocal(inv2[:], std2[:])
            nc.vector.scalar_tensor_tensor(
                nb2[:], mean2[:], -1.0, inv2[:], op0=ALU.mult, op1=ALU.mult
            )

            for f in range(FN):
                fsl = slice(f * 512, (f + 1) * 512)
                nc.scalar.activation(
                    y2[:, fsl], y2[:, fsl], ACTF.Relu, bias=nb2[:], scale=inv2[:]
                )
                ot = sml.tile([P, 4, C2O], F32, tag="ot", bufs=2)
                for q in range(4):
                    nch = f * 4 + q
                    pst = tpp.tile([P, P], F32, tag="tp")
                    nc.tensor.transpose(
                        pst[:], y2[:, nch * P:(nch + 1) * P], ident[:]
                    )
                    nc.any.tensor_copy(ot[:, q, :], pst[:])
                nc.sync.dma_start(
                    out_d[f * 512:(f + 1) * 512, :].rearrange(
                        "(q p) o -> p q o", p=P
                    ),
                    ot[:],
                )

    nc.compile()
    return nc


_NC_CACHE = {}


def _get_nc(key=(8192, 2048)):
    if key not in _NC_CACHE:
        _NC_CACHE[key] = build_nc(N=key[0], S=key[1])
    return _NC_CACHE[key]


def run(inputs, trace=False, **kw):
    from concourse.bass_utils import run_bass_kernel_spmd

    B = inputs["xyz1"].shape[0]
    assert B == 8
    nc = _get_nc()
    names = ("xyz1", "xyz2", "points1", "points2")
    in_maps = []
    for b in range(B):
        m = {k: np.ascontiguousarray(inputs[k][b], dtype=np.float32) for k in names}
        m["W1"] = np.ascontiguousarray(inputs["W1"], dtype=np.float32)
        m["W2"] = np.ascontiguousarray(inputs["W2"], dtype=np.float32)
        in_maps.append(m)
    return run_bass_kernel_spmd(
        nc, in_maps, core_ids=list(range(B)), trace=trace, **kw
    )


def kernel(**inputs):
    res = run(inputs)
    return np.stack([res.results[b]["out"] for b in range(8)])
